# revision 30
# baseline (speedup 1.0000x reference)
"""GatheringLoss on 8 Trainium2 NeuronCores.

queries (8, 4096, 512) f32, items (1024, 512) f32 -> loss (8, 4096) f32

Reference pipeline: rfft(queries, axis=S) -> phase-only spectrum
(conj(F)/|F|) -> irfft -> score = unit @ items.T -> argmax_k ->
loss = ||q - items[argmax]||^2 summed over F.

Device mapping (data-parallel over batch, one batch element per core):
  - rfft/irfft are expressed as dense DFT matmuls against precomputed
    cos/sin matrices (fp16, fp32 PSUM accumulation) on the TensorEngine.
  - phase normalization (x / |F|) on ACT (sqrt) + DVE (reciprocal, mult).
  - the inverse DFT directly produces unit^T (F x S layout) so the score
    matmul needs no transpose.
  - argmax + gather are replaced by: loss = ||q||^2 - 2 * sel(score2)
    where score2 = q @ items.T - ||items||^2 / 2 and sel picks the entry
    at argmax_k(score) via an is_equal mask (fused mask*select*reduce in
    one DVE scalar_tensor_tensor with accum_out).
Queries travel to the device as fp16 (halves the axon-tunnel bytes, the
dominant per-call cost); measured end-to-end rel err 2.455e-3 vs the
2e-2 gate.  The DFT matrices and itemsT ship to core 0 only and are
replicated on-device by an AllReduce against device-created zero shards
on cores 1-7 (NeuronLink speed instead of 8x tunnel bytes).  All
program build + walrus compile + DFT upload happens at import;
kernel() ships q16 (32MB), runs, and fetches the (8,4096) f32 loss.
"""

import hashlib
import os
import time

import numpy as np

B, S, F, K = 8, 4096, 512, 1024
J = S // 2 + 1  # 2049 rfft bins
JP = 2176  # padded to 17*128
NCORES = 8

_STATE = {}


def _build_dft_matrices():
    """fp16 DFT matrices. CF/SF: (S, JP) forward cos/sin (lhsT layout).
    IC/IS: (JP, S) inverse, with irfft weights folded in, IS negated."""
    s = np.arange(S, dtype=np.int64)
    j = np.arange(JP, dtype=np.int64)
    sj = (s[:, None] * j[None, :]) % S
    ang = (2.0 * np.pi / S) * sj
    C = np.cos(ang)
    Sn = np.sin(ang)
    C[:, J:] = 0.0
    Sn[:, J:] = 0.0
    w = np.zeros(JP)
    w[:J] = 2.0
    w[0] = 1.0
    w[J - 1] = 1.0
    CF = C.astype(np.float16)
    SF = Sn.astype(np.float16)
    IC = np.ascontiguousarray((C * w / S).T).astype(np.float16)
    IS = np.ascontiguousarray(-(Sn * w / S).T).astype(np.float16)
    return CF, SF, IC, IS


def _install_neff_disk_cache():
    """Memoize walrus (BIR -> NEFF) compiles on disk so fresh processes
    skip the compile when the program is unchanged."""
    import concourse.bass_utils as bu
    import concourse.bass2jax as b2j

    cache_dir = os.path.expanduser("~/.cache/bass_neff_cache")
    os.makedirs(cache_dir, exist_ok=True)
    orig = bu.compile_bir_kernel

    def cached(bir_json, tmpdir, neff_name="file.neff"):
        h = hashlib.sha256(
            bir_json if isinstance(bir_json, bytes) else bir_json.encode()
        ).hexdigest()[:32]
        path = os.path.join(cache_dir, f"{h}.neff")
        out_path = os.path.join(tmpdir, neff_name)
        if os.path.exists(path):
            with open(path, "rb") as f:
                data = f.read()
            with open(out_path, "wb") as f:
                f.write(data)
            return out_path
        r = orig(bir_json, tmpdir, neff_name)
        try:
            with open(r, "rb") as f:
                data = f.read()
            tmp = path + ".tmp"
            with open(tmp, "wb") as f:
                f.write(data)
            os.replace(tmp, path)
        except OSError:
            pass
        return r

    bu.compile_bir_kernel = cached
    b2j.compile_bir_kernel = cached


def _build_program():
    """Per-core Bass/Tile program."""
    import concourse.bass as bass  # noqa: F401
    import concourse.mybir as mybir
    from concourse import bacc
    from concourse.tile import TileContext

    f16 = mybir.dt.float16
    f32 = mybir.dt.float32
    Alu = mybir.AluOpType
    Act = mybir.ActivationFunctionType
    Ax = mybir.AxisListType

    nc = bacc.Bacc("TRN2", target_bir_lowering=False, debug=False, num_devices=NCORES)

    q_d = nc.declare_dram_parameter("q16", [S, F], f16, isOutput=False)
    itT_d = nc.declare_dram_parameter("itemsT", [F, K], f16, isOutput=False)
    nesq_d = nc.declare_dram_parameter("negesqhalf", [1, K], f16, isOutput=False)
    CF_d = nc.declare_dram_parameter("CF", [S, JP], f16, isOutput=False)
    SF_d = nc.declare_dram_parameter("SF", [S, JP], f16, isOutput=False)
    IC_d = nc.declare_dram_parameter("IC", [JP, S], f16, isOutput=False)
    IS_d = nc.declare_dram_parameter("IS", [JP, S], f16, isOutput=False)
    loss_d = nc.declare_dram_parameter("loss", [S, 1], f32, isOutput=True)

    NSC = S // 128  # 32 s-chunks
    NJC = JP // 128  # 17 j-chunks
    NFC = F // 128  # 4 f-chunks
    NTR = S // 512  # 8 t-ranges
    QUADS = [(0, 512), (512, 512), (1024, 512), (1536, 512), (2048, 128)]

    with TileContext(nc) as tc:
        with (
            tc.tile_pool(name="res", bufs=1) as res,
            tc.tile_pool(name="blk", bufs=8) as blk,
            tc.tile_pool(name="scr", bufs=2) as scr,
            tc.tile_pool(name="jnk", bufs=4) as jnk,
            tc.tile_pool(name="sml", bufs=4) as sml,
            tc.tile_pool(name="dram", bufs=1, space="DRAM") as dram,
            tc.tile_pool(name="ps", bufs=8, space="PSUM") as ps,
        ):
            # ---- on-device replication of host-shipped-once tensors ----
            # Only core 0 receives real data (cores 1-7 get persistent
            # device-resident zeros), so an AllReduce(add) replicates at
            # NeuronLink speed instead of 8x over the axon tunnel.
            def cc_replicate(src_ap, shape, nm):
                bin_ = dram.tile(shape, f16, tag=f"cin_{nm}", name=f"cin_{nm}")
                bout = dram.tile(
                    shape, f16, tag=f"cout_{nm}", name=f"cout_{nm}",
                    addr_space="Shared",
                )
                nc.gpsimd.dma_start(bin_[:], src_ap[:])
                nc.gpsimd.collective_compute(
                    "AllReduce", Alu.add,
                    replica_groups=[list(range(NCORES))],
                    ins=[bin_.opt()], outs=[bout.opt()],
                )
                return bout

            itT_r = cc_replicate(itT_d, [F, K], "it")
            CF_r = cc_replicate(CF_d, [S, JP], "cf")
            SF_r = cc_replicate(SF_d, [S, JP], "sf")
            IC_r = cc_replicate(IC_d, [JP, S], "ic")
            IS_r = cc_replicate(IS_d, [JP, S], "is")

            # ---- resident tensors ----
            qs = res.tile([128, NSC, 512], f16, tag="qs")
            nc.sync.dma_start(qs[:], q_d.rearrange("(n p) f -> p n f", p=128))
            itk = res.tile([128, NFC, K], f16, tag="itk")
            nc.sync.dma_start(itk[:], itT_r.rearrange("(c p) k -> p c k", p=128))
            nesq = res.tile([1, K], f16, tag="nesq")
            nc.sync.dma_start(nesq[:], nesq_d[:])
            ones = res.tile([1, 128], f16, tag="ones")
            nc.vector.memset(ones[:], 1.0)
            qsq = res.tile([128, NSC], f32, tag="qsq")
            ure = res.tile([128, NJC, 512], f16, tag="ure")
            vim = res.tile([128, NJC, 512], f16, tag="vim")
            ut = res.tile([128, NFC, S], f16, tag="ut")
            qt = res.tile([128, NFC, S], f16, tag="qt")

            # q^T via DMA transpose (for the q @ items.T matmul)
            for fc in range(NFC):
                nc.sync.dma_start_transpose(
                    qt[:, fc, :], q_d[:, fc * 128 : (fc + 1) * 128]
                )

            # ||q||^2 per row via ACT Square with accumulate
            for sc in range(NSC):
                junk = jnk.tile([128, 512], f16, tag="junk")
                nc.scalar.activation(
                    junk[:], qs[:, sc, :], Act.Square,
                    accum_out=qsq[:, sc : sc + 1],
                )

            # ---- forward DFT:  Fre/Fim_neg (JP, F) ----
            for j0, w in QUADS:
                njc = w // 128
                pc = [ps.tile([128, 512], f32, tag="ps", name=f"pc{j0}_{i}") for i in range(njc)]
                psn = [ps.tile([128, 512], f32, tag="ps", name=f"psn{j0}_{i}") for i in range(njc)]
                for sc in range(NSC):
                    cfb = blk.tile([128, w], f16, tag="blkA")
                    sfb = blk.tile([128, w], f16, tag="blkB")
                    nc.sync.dma_start(
                        cfb[:, :w], CF_r[sc * 128 : (sc + 1) * 128, j0 : j0 + w]
                    )
                    nc.sync.dma_start(
                        sfb[:, :w], SF_r[sc * 128 : (sc + 1) * 128, j0 : j0 + w]
                    )
                    for c in range(njc):
                        nc.tensor.matmul(
                            pc[c][:], cfb[:, c * 128 : (c + 1) * 128], qs[:, sc, :],
                            start=(sc == 0), stop=(sc == NSC - 1),
                        )
                        nc.tensor.matmul(
                            psn[c][:], sfb[:, c * 128 : (c + 1) * 128], qs[:, sc, :],
                            start=(sc == 0), stop=(sc == NSC - 1),
                        )
                # phase normalize this quad -> ure/vim (fp16)
                for c in range(njc):
                    jc = j0 // 128 + c
                    fre = scr.tile([128, 512], f32, tag="fre")
                    fim = scr.tile([128, 512], f32, tag="fim")
                    nc.scalar.activation(fre[:], pc[c][:], Act.Copy)
                    nc.scalar.activation(fim[:], psn[c][:], Act.Copy)
                    sq = scr.tile([128, 512], f32, tag="sq")
                    nc.scalar.activation(sq[:], fre[:], Act.Square)
                    sq2 = scr.tile([128, 512], f32, tag="sq2")
                    nc.scalar.activation(sq2[:], fim[:], Act.Square)
                    ssum = scr.tile([128, 512], f32, tag="ssum")
                    nc.vector.tensor_tensor(ssum[:], sq[:], sq2[:], Alu.add)
                    ssc = scr.tile([128, 512], f32, tag="ssc")
                    nc.vector.tensor_scalar(ssc[:], ssum[:], 1e-12, None, op0=Alu.max)
                    mag = scr.tile([128, 512], f32, tag="mag")
                    nc.scalar.activation(mag[:], ssc[:], Act.Sqrt)
                    rinv = scr.tile([128, 512], f32, tag="rinv")
                    nc.vector.reciprocal(rinv[:], mag[:])
                    nc.vector.tensor_tensor(ure[:, jc, :], fre[:], rinv[:], Alu.mult)
                    nc.vector.tensor_tensor(vim[:, jc, :], fim[:], rinv[:], Alu.mult)

            # ---- inverse DFT -> unit^T (F, S) ----
            for tr in range(NTR):
                pu = [ps.tile([128, 512], f32, tag="ps", name=f"pu{tr}_{i}") for i in range(NFC)]
                for jc in range(NJC):
                    icb = blk.tile([128, 512], f16, tag="blkA")
                    isb = blk.tile([128, 512], f16, tag="blkB")
                    nc.sync.dma_start(
                        icb[:], IC_r[jc * 128 : (jc + 1) * 128, tr * 512 : (tr + 1) * 512]
                    )
                    nc.sync.dma_start(
                        isb[:], IS_r[jc * 128 : (jc + 1) * 128, tr * 512 : (tr + 1) * 512]
                    )
                    for fc in range(NFC):
                        nc.tensor.matmul(
                            pu[fc][:], ure[:, jc, fc * 128 : (fc + 1) * 128], icb[:],
                            start=(jc == 0), stop=False,
                        )
                        nc.tensor.matmul(
                            pu[fc][:], vim[:, jc, fc * 128 : (fc + 1) * 128], isb[:],
                            start=False, stop=(jc == NJC - 1),
                        )
                for fc in range(NFC):
                    nc.scalar.activation(
                        ut[:, fc, tr * 512 : (tr + 1) * 512], pu[fc][:], Act.Copy
                    )

            # ---- score, score2, argmax-select, loss ----
            for sc in range(NSC):
                pss = [ps.tile([128, 512], f32, tag="ps", name=f"pss{sc}_{i}") for i in range(2)]
                ps2 = [ps.tile([128, 512], f32, tag="ps", name=f"ps2{sc}_{i}") for i in range(2)]
                for kh in range(2):
                    for fc in range(NFC):
                        nc.tensor.matmul(
                            pss[kh][:],
                            ut[:, fc, sc * 128 : (sc + 1) * 128],
                            itk[:, fc, kh * 512 : (kh + 1) * 512],
                            start=(fc == 0), stop=(fc == NFC - 1),
                        )
                    for fc in range(NFC):
                        nc.tensor.matmul(
                            ps2[kh][:],
                            qt[:, fc, sc * 128 : (sc + 1) * 128],
                            itk[:, fc, kh * 512 : (kh + 1) * 512],
                            start=(fc == 0), stop=False,
                        )
                    nc.tensor.matmul(
                        ps2[kh][:], ones[:], nesq[:, kh * 512 : (kh + 1) * 512],
                        start=False, stop=True,
                    )
                mx = sml.tile([128, 4], f32, tag="mx")
                for kh in range(2):
                    nc.vector.tensor_reduce(
                        mx[:, kh : kh + 1], pss[kh][:], Ax.X, Alu.max
                    )
                nc.vector.tensor_tensor(
                    mx[:, 2:3], mx[:, 0:1], mx[:, 1:2], Alu.max
                )
                s2s = [jnk.tile([128, 512], f16, tag="s2s", name=f"s2s{sc}_{i}") for i in range(2)]
                for kh in range(2):
                    nc.scalar.activation(s2s[kh][:], ps2[kh][:], Act.Copy)
                acc = sml.tile([128, 8], f32, tag="acc")
                for kh in range(2):
                    junk = jnk.tile([128, 512], f16, tag="junkm")
                    nc.vector.scalar_tensor_tensor(
                        junk[:], pss[kh][:], mx[:, 2:3], s2s[kh][:],
                        op0=Alu.is_equal, op1=Alu.mult,
                        accum_out=acc[:, kh : kh + 1],
                    )
                    junk2 = jnk.tile([128, 512], f16, tag="junkc")
                    nc.vector.tensor_scalar(
                        junk2[:], pss[kh][:], mx[:, 2:3], None,
                        op0=Alu.is_equal, op1=Alu.add,
                        accum_out=acc[:, 2 + kh : 3 + kh],
                    )
                nc.vector.tensor_tensor(
                    acc[:, 4:5], acc[:, 0:1], acc[:, 1:2], Alu.add
                )  # sel_sum
                nc.vector.tensor_tensor(
                    acc[:, 5:6], acc[:, 2:3], acc[:, 3:4], Alu.add
                )  # count
                nc.vector.reciprocal(acc[:, 6:7], acc[:, 5:6])
                nc.vector.tensor_tensor(
                    acc[:, 7:8], acc[:, 4:5], acc[:, 6:7], Alu.mult
                )  # sel
                lossT = sml.tile([128, 1], f32, tag="lossT")
                nc.vector.scalar_tensor_tensor(
                    lossT[:], acc[:, 7:8], -2.0, qsq[:, sc : sc + 1],
                    op0=Alu.mult, op1=Alu.add,
                )
                nc.sync.dma_start(loss_d[sc * 128 : (sc + 1) * 128, :], lossT[:])

    nc.compile()
    return nc


def _make_runner(nc):
    """Persistent jit'd SPMD callable mirroring bass2jax.run_bass_via_pjrt."""
    import jax
    import jax.numpy as jnp  # noqa: F401
    from jax.experimental.shard_map import shard_map
    from jax.sharding import Mesh, NamedSharding, PartitionSpec

    import concourse.mybir as mybir
    from concourse import bass2jax as b2j

    b2j.install_neuronx_cc_hook()

    partition_name = nc.partition_id_tensor.name if nc.partition_id_tensor else None
    in_names = []
    out_names = []
    out_avals = []
    zero_outs = []
    for alloc in nc.m.functions[0].allocations:
        if not isinstance(alloc, mybir.MemoryLocationSet):
            continue
        name = alloc.memorylocations[0].name
        if alloc.kind == "ExternalInput":
            if name != partition_name:
                in_names.append(name)
        elif alloc.kind == "ExternalOutput":
            out_names.append(name)
            shape = tuple(alloc.tensor_shape)
            dtype = mybir.dt.np(alloc.dtype)
            out_avals.append(jax.core.ShapedArray(shape, dtype))
            zero_outs.append(np.zeros(shape, dtype))
    n_params = len(in_names)
    n_outs = len(out_avals)
    all_names = in_names + out_names
    if partition_name is not None:
        all_names = all_names + [partition_name]
    donate = tuple(range(n_params, n_params + n_outs))

    def _body(*args):
        operands = list(args)
        if partition_name is not None:
            operands.append(b2j.partition_id_tensor())
        outs = b2j._bass_exec_p.bind(
            *operands,
            out_avals=tuple(out_avals),
            in_names=tuple(all_names),
            out_names=tuple(out_names),
            lowering_input_output_aliases=(),
            sim_require_finite=True,
            sim_require_nnan=True,
            nc=nc,
        )
        return tuple(outs)

    devices = jax.devices()[:NCORES]
    assert len(devices) == NCORES
    mesh = Mesh(np.asarray(devices), ("core",))
    spec = NamedSharding(mesh, PartitionSpec("core"))
    in_specs = (PartitionSpec("core"),) * (n_params + n_outs)
    out_specs = (PartitionSpec("core"),) * n_outs

    def jit_factory():
        return jax.jit(
            shard_map(
                _body, mesh=mesh, in_specs=in_specs, out_specs=out_specs,
                check_rep=False,
            ),
            donate_argnums=donate,
            keep_unused=True,
        )

    return jit_factory, in_names, out_names, spec, zero_outs


def _host_kernel(queries: np.ndarray, items: np.ndarray) -> np.ndarray:
    """Pure-numpy fallback (used only if the axon TRN devices are absent)."""
    queries = np.asarray(queries, dtype=np.float32)
    items = np.asarray(items, dtype=np.float32)
    f = np.fft.rfft(queries, axis=1)
    ang = np.angle(f)
    unit = np.fft.irfft(np.cos(ang) - 1j * np.sin(ang), axis=1, n=S).astype(
        np.float32
    )
    out = np.empty((queries.shape[0], queries.shape[1]), dtype=np.float32)
    for b in range(queries.shape[0]):
        score = unit[b] @ items.T
        idx = np.argmax(score, axis=-1)
        d = queries[b] - items[idx]
        out[b] = np.sum(d * d, axis=-1)
    return out


def _devices_ok():
    try:
        import jax

        devs = jax.devices()
        return len(devs) >= NCORES and devs[0].platform != "cpu"
    except Exception:
        return False


def _ensure_init():
    if "fn" in _STATE or _STATE.get("fallback"):
        return
    if not _devices_ok():
        _STATE["fallback"] = True
        return
    t0 = time.perf_counter()
    _install_neff_disk_cache()
    nc = _build_program()
    t1 = time.perf_counter()
    jit_factory, in_names, out_names, spec, zero_outs = _make_runner(nc)
    t2 = time.perf_counter()

    import jax
    import jax.numpy as jnp

    CF, SF, IC, IS = _build_dft_matrices()

    from concurrent.futures import ThreadPoolExecutor

    devices = jax.devices()[:NCORES]
    _STATE.update(
        in_names=in_names, out_names=out_names, spec=spec,
        zero_outs=zero_outs, nc=nc,
        devices=devices,
        pool=ThreadPoolExecutor(NCORES),
    )

    def rep0(a):
        """Shard array: real data on core 0, device-created zeros on 1-7.
        The NEFF AllReduce-replicates at NeuronLink speed."""
        z = jnp.zeros((NCORES * a.shape[0],) + a.shape[1:], a.dtype, device=spec)
        shards = sorted(z.addressable_shards, key=lambda s: s.index[0].start)
        real0 = jax.device_put(a, devices[0])
        return jax.make_array_from_single_device_arrays(
            z.shape, spec, [real0] + [s.data for s in shards[1:]]
        )

    dft_dev = {
        "CF": rep0(CF), "SF": rep0(SF), "IC": rep0(IC), "IS": rep0(IS),
    }
    for v in dft_dev.values():
        v.block_until_ready()
    _STATE["dft"] = dft_dev
    _STATE["rep0"] = rep0
    t3 = time.perf_counter()

    # AOT-compile on the fast dispatch path, then warm with zero inputs
    from concourse import bass2jax as b2j

    q0 = jax.device_put(np.zeros((NCORES * S, F), np.float16), spec)
    it0 = jax.device_put(np.zeros((NCORES * F, K), np.float16), spec)
    ne0 = jax.device_put(np.zeros((NCORES * 1, K), np.float16), spec)
    arrs = {"q16": q0, "itemsT": it0, "negesqhalf": ne0, **dft_dev}
    args = [arrs[n] for n in in_names]
    zeros = [
        jax.device_put(
            np.zeros((NCORES * z.shape[0],) + z.shape[1:], z.dtype), spec
        )
        for z in zero_outs
    ]
    fn = b2j.fast_dispatch_compile(
        lambda: jit_factory().lower(*args, *zeros).compile()
    )
    _STATE["fn"] = fn
    out = fn(*args, *zeros)
    jax.block_until_ready(out)
    del out
    t4 = time.perf_counter()
    if os.environ.get("KERNEL_DEBUG"):
        print(
            f"[kernel init] build={t1-t0:.1f}s runner={t2-t1:.1f}s "
            f"dft_upload={t3-t2:.1f}s warmup={t4-t3:.1f}s",
            flush=True,
        )


def _call_device(q16_dev, itT_dev, nesq_dev, zeros=None):
    import jax

    st = _STATE
    arrs = {
        "q16": q16_dev,
        "itemsT": itT_dev,
        "negesqhalf": nesq_dev,
        **st["dft"],
    }
    args = [arrs[n] for n in st["in_names"]]
    if zeros is None:
        zeros = [
            jax.device_put(
                np.zeros((NCORES * z.shape[0],) + z.shape[1:], z.dtype), st["spec"]
            )
            for z in st["zero_outs"]
        ]
    outs = st["fn"](*args, *zeros)
    return outs


def kernel(queries: np.ndarray, items: np.ndarray) -> np.ndarray:
    import traceback

    try:
        _ensure_init()
    except Exception:
        traceback.print_exc()
        _STATE["fallback"] = True
    if _STATE.get("fallback"):
        return _host_kernel(queries, items)
    try:
        return _kernel_device(queries, items)
    except Exception:
        traceback.print_exc()
    try:
        # one retry: transient transfer/exec hiccups shouldn't demote us
        # to the slow host path permanently
        return _kernel_device(queries, items)
    except Exception:
        traceback.print_exc()
        _STATE["fallback"] = True
        return _host_kernel(queries, items)


def _kernel_device(queries: np.ndarray, items: np.ndarray) -> np.ndarray:
    import jax

    dbg = os.environ.get("KERNEL_DEBUG")
    t0 = time.perf_counter()
    st = _STATE
    spec = st["spec"]
    devices = st["devices"]
    pool = st["pool"]

    # Issue small tensors first (their handshakes shouldn't tail behind
    # the 32MB q transfer), then q per-batch so chunk 0's transfer
    # overlaps the remaining fp32->fp16 conversions.
    zeros = [
        jax.device_put(
            np.zeros((NCORES * z.shape[0],) + z.shape[1:], z.dtype), spec
        )
        for z in st["zero_outs"]
    ]
    items_key = hashlib.sha1(np.ascontiguousarray(items)).hexdigest()
    cached = st.get("items_cache")
    if cached is not None and cached[0] == items_key:
        itT_dev, ne_dev = cached[1], cached[2]
    else:
        itemsf = np.asarray(items, np.float32)
        itT = np.ascontiguousarray(itemsf.T).astype(np.float16)
        esq = np.sum(itemsf * itemsf, axis=1)
        nesq = (-0.5 * esq).astype(np.float16)[None, :]
        itT_dev = st["rep0"](itT)  # core 0 only; NEFF AllReduce replicates
        ne_dev = jax.device_put(
            np.broadcast_to(nesq, (NCORES,) + nesq.shape).reshape(NCORES, K), spec
        )
        st["items_cache"] = (items_key, itT_dev, ne_dev)
    q = np.asarray(queries)
    shards = []
    for b in range(NCORES):
        qb = np.asarray(q[b], np.float32).astype(np.float16)
        shards.append(jax.device_put(qb, devices[b]))
    q_dev = jax.make_array_from_single_device_arrays(
        (NCORES * S, F), spec, shards
    )
    t1 = time.perf_counter()
    outs = _call_device(q_dev, itT_dev, ne_dev, zeros)
    out_shards = sorted(outs[0].addressable_shards, key=lambda s: s.index[0].start)
    for sh in out_shards:
        try:
            sh.data.copy_to_host_async()
        except Exception:
            pass
    t2 = time.perf_counter()
    parts = list(pool.map(lambda s: np.asarray(s.data), out_shards))
    loss = np.concatenate(parts, axis=0).reshape(NCORES, S)
    t3 = time.perf_counter()
    if dbg:
        print(
            f"[kernel] issue={t1-t0:.3f}s call={t2-t1:.3f}s "
            f"fetch={t3-t2:.3f}s total={t3-t0:.3f}s",
            flush=True,
        )
    return loss


if not os.environ.get("KERNEL_NO_INIT"):
    try:
        _ensure_init()
    except Exception:
        import traceback

        traceback.print_exc()
        _STATE["fallback"] = True


# revision 34
# speedup vs baseline: 1.1413x; 1.1413x over previous
"""GatheringLoss on 8 Trainium2 NeuronCores.

queries (8, 4096, 512) f32, items (1024, 512) f32 -> loss (8, 4096) f32

Reference pipeline: rfft(queries, axis=S) -> phase-only spectrum
(conj(F)/|F|) -> irfft -> score = unit @ items.T -> argmax_k ->
loss = ||q - items[argmax]||^2 summed over F.

Device mapping (data-parallel over batch, one batch element per core):
  - rfft/irfft are expressed as dense DFT matmuls against precomputed
    cos/sin matrices (fp16, fp32 PSUM accumulation) on the TensorEngine.
  - phase normalization (x / |F|) on ACT (sqrt) + DVE (reciprocal, mult).
  - the inverse DFT directly produces unit^T (F x S layout) so the score
    matmul needs no transpose.
  - argmax + gather are replaced by: loss = ||q||^2 - 2 * sel(score2)
    where score2 = q @ items.T - ||items||^2 / 2 and sel picks the entry
    at argmax_k(score) via an is_equal mask (fused mask*select*reduce in
    one DVE scalar_tensor_tensor with accum_out).
Queries travel to the device as fp16 (halves the axon-tunnel bytes, the
dominant per-call cost); measured end-to-end rel err 2.455e-3 vs the
2e-2 gate.  The DFT matrices and itemsT ship to core 0 only and are
replicated on-device by an AllReduce against device-created zero shards
on cores 1-7 (NeuronLink speed instead of 8x tunnel bytes).  All
program build + walrus compile + DFT upload happens at import;
kernel() ships q16 (32MB), runs, and fetches the (8,4096) f32 loss.
"""

import hashlib
import os
import time

import numpy as np

B, S, F, K = 8, 4096, 512, 1024
J = S // 2 + 1  # 2049 rfft bins
JP = 2176  # padded to 17*128
NCORES = 8

_STATE = {}


def _build_dft_matrices():
    """fp16 DFT matrices. CF/SF: (S, JP) forward cos/sin (lhsT layout).
    IC/IS: (JP, S) inverse, with irfft weights folded in, IS negated."""
    s = np.arange(S, dtype=np.int64)
    j = np.arange(JP, dtype=np.int64)
    sj = (s[:, None] * j[None, :]) % S
    ang = (2.0 * np.pi / S) * sj
    C = np.cos(ang)
    Sn = np.sin(ang)
    C[:, J:] = 0.0
    Sn[:, J:] = 0.0
    w = np.zeros(JP)
    w[:J] = 2.0
    w[0] = 1.0
    w[J - 1] = 1.0
    CF = C.astype(np.float16)
    SF = Sn.astype(np.float16)
    IC = np.ascontiguousarray((C * w / S).T).astype(np.float16)
    IS = np.ascontiguousarray(-(Sn * w / S).T).astype(np.float16)
    return CF, SF, IC, IS


def _install_neff_disk_cache():
    """Memoize walrus (BIR -> NEFF) compiles on disk so fresh processes
    skip the compile when the program is unchanged."""
    import concourse.bass_utils as bu
    import concourse.bass2jax as b2j

    cache_dir = os.path.expanduser("~/.cache/bass_neff_cache")
    os.makedirs(cache_dir, exist_ok=True)
    orig = bu.compile_bir_kernel

    def cached(bir_json, tmpdir, neff_name="file.neff"):
        h = hashlib.sha256(
            bir_json if isinstance(bir_json, bytes) else bir_json.encode()
        ).hexdigest()[:32]
        path = os.path.join(cache_dir, f"{h}.neff")
        out_path = os.path.join(tmpdir, neff_name)
        if os.path.exists(path):
            with open(path, "rb") as f:
                data = f.read()
            with open(out_path, "wb") as f:
                f.write(data)
            return out_path
        r = orig(bir_json, tmpdir, neff_name)
        try:
            with open(r, "rb") as f:
                data = f.read()
            tmp = path + ".tmp"
            with open(tmp, "wb") as f:
                f.write(data)
            os.replace(tmp, path)
        except OSError:
            pass
        return r

    bu.compile_bir_kernel = cached
    b2j.compile_bir_kernel = cached


def _build_program():
    """Per-core Bass/Tile program."""
    import concourse.bass as bass  # noqa: F401
    import concourse.mybir as mybir
    from concourse import bacc
    from concourse.tile import TileContext

    f16 = mybir.dt.float16
    f32 = mybir.dt.float32
    Alu = mybir.AluOpType
    Act = mybir.ActivationFunctionType
    Ax = mybir.AxisListType

    nc = bacc.Bacc("TRN2", target_bir_lowering=False, debug=False, num_devices=NCORES)

    q_d = nc.declare_dram_parameter("q16", [S, F], f16, isOutput=False)
    itT_d = nc.declare_dram_parameter("itemsT", [F, K], f16, isOutput=False)
    nesq_d = nc.declare_dram_parameter("negesqhalf", [1, K], f16, isOutput=False)
    CF_d = nc.declare_dram_parameter("CF", [S, JP], f16, isOutput=False)
    SF_d = nc.declare_dram_parameter("SF", [S, JP], f16, isOutput=False)
    IC_d = nc.declare_dram_parameter("IC", [JP, S], f16, isOutput=False)
    IS_d = nc.declare_dram_parameter("IS", [JP, S], f16, isOutput=False)
    loss_d = nc.declare_dram_parameter("loss", [S, 1], f32, isOutput=True)

    NSC = S // 128  # 32 s-chunks
    NJC = JP // 128  # 17 j-chunks
    NFC = F // 128  # 4 f-chunks
    NTR = S // 512  # 8 t-ranges
    QUADS = [(0, 512), (512, 512), (1024, 512), (1536, 512), (2048, 128)]

    with TileContext(nc) as tc:
        with (
            tc.tile_pool(name="res", bufs=1) as res,
            tc.tile_pool(name="blk", bufs=8) as blk,
            tc.tile_pool(name="scr", bufs=2) as scr,
            tc.tile_pool(name="jnk", bufs=4) as jnk,
            tc.tile_pool(name="sml", bufs=4) as sml,
            tc.tile_pool(name="dram", bufs=1, space="DRAM") as dram,
            tc.tile_pool(name="ps", bufs=8, space="PSUM") as ps,
        ):
            # ---- on-device replication of host-shipped-once tensors ----
            # Only core 0 receives real data (cores 1-7 get persistent
            # device-resident zeros), so an AllReduce(add) replicates at
            # NeuronLink speed instead of 8x over the axon tunnel.
            def cc_replicate(src_ap, shape, nm):
                bin_ = dram.tile(shape, f16, tag=f"cin_{nm}", name=f"cin_{nm}")
                bout = dram.tile(
                    shape, f16, tag=f"cout_{nm}", name=f"cout_{nm}",
                    addr_space="Shared",
                )
                nc.gpsimd.dma_start(bin_[:], src_ap[:])
                nc.gpsimd.collective_compute(
                    "AllReduce", Alu.add,
                    replica_groups=[list(range(NCORES))],
                    ins=[bin_.opt()], outs=[bout.opt()],
                )
                return bout

            itT_r = cc_replicate(itT_d, [F, K], "it")
            # DFT matrices arrive pre-replicated (aux NEFF at import), so
            # the per-call program reads them directly.
            CF_r, SF_r, IC_r, IS_r = CF_d, SF_d, IC_d, IS_d

            # ---- resident tensors ----
            qs = res.tile([128, NSC, 512], f16, tag="qs")
            nc.sync.dma_start(qs[:], q_d.rearrange("(n p) f -> p n f", p=128))
            itk = res.tile([128, NFC, K], f16, tag="itk")
            nc.sync.dma_start(itk[:], itT_r.rearrange("(c p) k -> p c k", p=128))
            nesq = res.tile([1, K], f16, tag="nesq")
            nc.sync.dma_start(nesq[:], nesq_d[:])
            ones = res.tile([1, 128], f16, tag="ones")
            nc.vector.memset(ones[:], 1.0)
            qsq = res.tile([128, NSC], f32, tag="qsq")
            ure = res.tile([128, NJC, 512], f16, tag="ure")
            vim = res.tile([128, NJC, 512], f16, tag="vim")
            ut = res.tile([128, NFC, S], f16, tag="ut")
            qt = res.tile([128, NFC, S], f16, tag="qt")

            # q^T via DMA transpose (for the q @ items.T matmul)
            for fc in range(NFC):
                nc.sync.dma_start_transpose(
                    qt[:, fc, :], q_d[:, fc * 128 : (fc + 1) * 128]
                )

            # ||q||^2 per row via ACT Square with accumulate
            for sc in range(NSC):
                junk = jnk.tile([128, 512], f16, tag="junk")
                nc.scalar.activation(
                    junk[:], qs[:, sc, :], Act.Square,
                    accum_out=qsq[:, sc : sc + 1],
                )

            # ---- forward DFT:  Fre/Fim_neg (JP, F) ----
            for j0, w in QUADS:
                njc = w // 128
                pc = [ps.tile([128, 512], f32, tag="ps", name=f"pc{j0}_{i}") for i in range(njc)]
                psn = [ps.tile([128, 512], f32, tag="ps", name=f"psn{j0}_{i}") for i in range(njc)]
                for sc in range(NSC):
                    cfb = blk.tile([128, w], f16, tag="blkA")
                    sfb = blk.tile([128, w], f16, tag="blkB")
                    nc.sync.dma_start(
                        cfb[:, :w], CF_r[sc * 128 : (sc + 1) * 128, j0 : j0 + w]
                    )
                    nc.sync.dma_start(
                        sfb[:, :w], SF_r[sc * 128 : (sc + 1) * 128, j0 : j0 + w]
                    )
                    for c in range(njc):
                        nc.tensor.matmul(
                            pc[c][:], cfb[:, c * 128 : (c + 1) * 128], qs[:, sc, :],
                            start=(sc == 0), stop=(sc == NSC - 1),
                        )
                        nc.tensor.matmul(
                            psn[c][:], sfb[:, c * 128 : (c + 1) * 128], qs[:, sc, :],
                            start=(sc == 0), stop=(sc == NSC - 1),
                        )
                # phase normalize this quad -> ure/vim (fp16)
                for c in range(njc):
                    jc = j0 // 128 + c
                    fre = scr.tile([128, 512], f32, tag="fre")
                    fim = scr.tile([128, 512], f32, tag="fim")
                    nc.scalar.activation(fre[:], pc[c][:], Act.Copy)
                    nc.scalar.activation(fim[:], psn[c][:], Act.Copy)
                    sq = scr.tile([128, 512], f32, tag="sq")
                    nc.scalar.activation(sq[:], fre[:], Act.Square)
                    sq2 = scr.tile([128, 512], f32, tag="sq2")
                    nc.scalar.activation(sq2[:], fim[:], Act.Square)
                    ssum = scr.tile([128, 512], f32, tag="ssum")
                    nc.vector.tensor_tensor(ssum[:], sq[:], sq2[:], Alu.add)
                    ssc = scr.tile([128, 512], f32, tag="ssc")
                    nc.vector.tensor_scalar(ssc[:], ssum[:], 1e-12, None, op0=Alu.max)
                    mag = scr.tile([128, 512], f32, tag="mag")
                    nc.scalar.activation(mag[:], ssc[:], Act.Sqrt)
                    rinv = scr.tile([128, 512], f32, tag="rinv")
                    nc.vector.reciprocal(rinv[:], mag[:])
                    nc.vector.tensor_tensor(ure[:, jc, :], fre[:], rinv[:], Alu.mult)
                    nc.vector.tensor_tensor(vim[:, jc, :], fim[:], rinv[:], Alu.mult)

            # ---- inverse DFT -> unit^T (F, S) ----
            for tr in range(NTR):
                pu = [ps.tile([128, 512], f32, tag="ps", name=f"pu{tr}_{i}") for i in range(NFC)]
                for jc in range(NJC):
                    icb = blk.tile([128, 512], f16, tag="blkA")
                    isb = blk.tile([128, 512], f16, tag="blkB")
                    nc.sync.dma_start(
                        icb[:], IC_r[jc * 128 : (jc + 1) * 128, tr * 512 : (tr + 1) * 512]
                    )
                    nc.sync.dma_start(
                        isb[:], IS_r[jc * 128 : (jc + 1) * 128, tr * 512 : (tr + 1) * 512]
                    )
                    for fc in range(NFC):
                        nc.tensor.matmul(
                            pu[fc][:], ure[:, jc, fc * 128 : (fc + 1) * 128], icb[:],
                            start=(jc == 0), stop=False,
                        )
                        nc.tensor.matmul(
                            pu[fc][:], vim[:, jc, fc * 128 : (fc + 1) * 128], isb[:],
                            start=False, stop=(jc == NJC - 1),
                        )
                for fc in range(NFC):
                    nc.scalar.activation(
                        ut[:, fc, tr * 512 : (tr + 1) * 512], pu[fc][:], Act.Copy
                    )

            # ---- score, score2, argmax-select, loss ----
            for sc in range(NSC):
                pss = [ps.tile([128, 512], f32, tag="ps", name=f"pss{sc}_{i}") for i in range(2)]
                ps2 = [ps.tile([128, 512], f32, tag="ps", name=f"ps2{sc}_{i}") for i in range(2)]
                for kh in range(2):
                    for fc in range(NFC):
                        nc.tensor.matmul(
                            pss[kh][:],
                            ut[:, fc, sc * 128 : (sc + 1) * 128],
                            itk[:, fc, kh * 512 : (kh + 1) * 512],
                            start=(fc == 0), stop=(fc == NFC - 1),
                        )
                    for fc in range(NFC):
                        nc.tensor.matmul(
                            ps2[kh][:],
                            qt[:, fc, sc * 128 : (sc + 1) * 128],
                            itk[:, fc, kh * 512 : (kh + 1) * 512],
                            start=(fc == 0), stop=False,
                        )
                    nc.tensor.matmul(
                        ps2[kh][:], ones[:], nesq[:, kh * 512 : (kh + 1) * 512],
                        start=False, stop=True,
                    )
                mx = sml.tile([128, 4], f32, tag="mx")
                for kh in range(2):
                    nc.vector.tensor_reduce(
                        mx[:, kh : kh + 1], pss[kh][:], Ax.X, Alu.max
                    )
                nc.vector.tensor_tensor(
                    mx[:, 2:3], mx[:, 0:1], mx[:, 1:2], Alu.max
                )
                s2s = [jnk.tile([128, 512], f16, tag="s2s", name=f"s2s{sc}_{i}") for i in range(2)]
                for kh in range(2):
                    nc.scalar.activation(s2s[kh][:], ps2[kh][:], Act.Copy)
                acc = sml.tile([128, 8], f32, tag="acc")
                for kh in range(2):
                    junk = jnk.tile([128, 512], f16, tag="junkm")
                    nc.vector.scalar_tensor_tensor(
                        junk[:], pss[kh][:], mx[:, 2:3], s2s[kh][:],
                        op0=Alu.is_equal, op1=Alu.mult,
                        accum_out=acc[:, kh : kh + 1],
                    )
                    junk2 = jnk.tile([128, 512], f16, tag="junkc")
                    nc.vector.tensor_scalar(
                        junk2[:], pss[kh][:], mx[:, 2:3], None,
                        op0=Alu.is_equal, op1=Alu.add,
                        accum_out=acc[:, 2 + kh : 3 + kh],
                    )
                nc.vector.tensor_tensor(
                    acc[:, 4:5], acc[:, 0:1], acc[:, 1:2], Alu.add
                )  # sel_sum
                nc.vector.tensor_tensor(
                    acc[:, 5:6], acc[:, 2:3], acc[:, 3:4], Alu.add
                )  # count
                nc.vector.reciprocal(acc[:, 6:7], acc[:, 5:6])
                nc.vector.tensor_tensor(
                    acc[:, 7:8], acc[:, 4:5], acc[:, 6:7], Alu.mult
                )  # sel
                lossT = sml.tile([128, 1], f32, tag="lossT")
                nc.vector.scalar_tensor_tensor(
                    lossT[:], acc[:, 7:8], -2.0, qsq[:, sc : sc + 1],
                    op0=Alu.mult, op1=Alu.add,
                )
                nc.sync.dma_start(loss_d[sc * 128 : (sc + 1) * 128, :], lossT[:])

    nc.compile()
    return nc


def _build_replicator():
    """Aux program: AllReduce the 4 DFT matrices (core 0 real, others
    zero) into per-core replicas, written to outputs that stay
    device-resident.  Runs once at import."""
    import concourse.mybir as mybir
    from concourse import bacc
    from concourse.tile import TileContext

    f16 = mybir.dt.float16
    Alu = mybir.AluOpType
    nc = bacc.Bacc("TRN2", target_bir_lowering=False, debug=False, num_devices=NCORES)
    shapes = {"CF": [S, JP], "SF": [S, JP], "IC": [JP, S], "IS": [JP, S]}
    ins = {
        nm: nc.declare_dram_parameter(nm, sh, f16, isOutput=False)
        for nm, sh in shapes.items()
    }
    outs = {
        nm: nc.declare_dram_parameter(nm + "r", sh, f16, isOutput=True)
        for nm, sh in shapes.items()
    }
    with TileContext(nc) as tc:
        with tc.tile_pool(name="dram", bufs=1, space="DRAM") as dram:
            for nm, sh in shapes.items():
                bin_ = dram.tile(sh, f16, tag=f"i{nm}", name=f"i{nm}")
                bout = dram.tile(
                    sh, f16, tag=f"o{nm}", name=f"o{nm}", addr_space="Shared"
                )
                nc.gpsimd.dma_start(bin_[:], ins[nm][:])
                nc.gpsimd.collective_compute(
                    "AllReduce", Alu.add,
                    replica_groups=[list(range(NCORES))],
                    ins=[bin_.opt()], outs=[bout.opt()],
                )
                nc.gpsimd.dma_start(outs[nm][:], bout[:])
    nc.compile()
    return nc


def _make_runner(nc):
    """Persistent jit'd SPMD callable mirroring bass2jax.run_bass_via_pjrt."""
    import jax
    import jax.numpy as jnp  # noqa: F401
    from jax.experimental.shard_map import shard_map
    from jax.sharding import Mesh, NamedSharding, PartitionSpec

    import concourse.mybir as mybir
    from concourse import bass2jax as b2j

    b2j.install_neuronx_cc_hook()

    partition_name = nc.partition_id_tensor.name if nc.partition_id_tensor else None
    in_names = []
    out_names = []
    out_avals = []
    zero_outs = []
    for alloc in nc.m.functions[0].allocations:
        if not isinstance(alloc, mybir.MemoryLocationSet):
            continue
        name = alloc.memorylocations[0].name
        if alloc.kind == "ExternalInput":
            if name != partition_name:
                in_names.append(name)
        elif alloc.kind == "ExternalOutput":
            out_names.append(name)
            shape = tuple(alloc.tensor_shape)
            dtype = mybir.dt.np(alloc.dtype)
            out_avals.append(jax.core.ShapedArray(shape, dtype))
            zero_outs.append(np.zeros(shape, dtype))
    n_params = len(in_names)
    n_outs = len(out_avals)
    all_names = in_names + out_names
    if partition_name is not None:
        all_names = all_names + [partition_name]
    donate = tuple(range(n_params, n_params + n_outs))

    def _body(*args):
        operands = list(args)
        if partition_name is not None:
            operands.append(b2j.partition_id_tensor())
        outs = b2j._bass_exec_p.bind(
            *operands,
            out_avals=tuple(out_avals),
            in_names=tuple(all_names),
            out_names=tuple(out_names),
            lowering_input_output_aliases=(),
            sim_require_finite=True,
            sim_require_nnan=True,
            nc=nc,
        )
        return tuple(outs)

    devices = jax.devices()[:NCORES]
    assert len(devices) == NCORES
    mesh = Mesh(np.asarray(devices), ("core",))
    spec = NamedSharding(mesh, PartitionSpec("core"))
    in_specs = (PartitionSpec("core"),) * (n_params + n_outs)
    out_specs = (PartitionSpec("core"),) * n_outs

    def jit_factory():
        return jax.jit(
            shard_map(
                _body, mesh=mesh, in_specs=in_specs, out_specs=out_specs,
                check_rep=False,
            ),
            donate_argnums=donate,
            keep_unused=True,
        )

    return jit_factory, in_names, out_names, spec, zero_outs


def _host_kernel(queries: np.ndarray, items: np.ndarray) -> np.ndarray:
    """Pure-numpy fallback (used only if the axon TRN devices are absent)."""
    queries = np.asarray(queries, dtype=np.float32)
    items = np.asarray(items, dtype=np.float32)
    f = np.fft.rfft(queries, axis=1)
    ang = np.angle(f)
    unit = np.fft.irfft(np.cos(ang) - 1j * np.sin(ang), axis=1, n=S).astype(
        np.float32
    )
    out = np.empty((queries.shape[0], queries.shape[1]), dtype=np.float32)
    for b in range(queries.shape[0]):
        score = unit[b] @ items.T
        idx = np.argmax(score, axis=-1)
        d = queries[b] - items[idx]
        out[b] = np.sum(d * d, axis=-1)
    return out


def _devices_ok():
    try:
        import jax

        devs = jax.devices()
        return len(devs) >= NCORES and devs[0].platform != "cpu"
    except Exception:
        return False


def _ensure_init():
    if "fn" in _STATE or _STATE.get("fallback"):
        return
    if not _devices_ok():
        _STATE["fallback"] = True
        return
    t0 = time.perf_counter()
    _install_neff_disk_cache()
    nc = _build_program()
    t1 = time.perf_counter()
    jit_factory, in_names, out_names, spec, zero_outs = _make_runner(nc)
    t2 = time.perf_counter()

    import jax
    import jax.numpy as jnp

    CF, SF, IC, IS = _build_dft_matrices()

    from concurrent.futures import ThreadPoolExecutor

    devices = jax.devices()[:NCORES]
    _STATE.update(
        in_names=in_names, out_names=out_names, spec=spec,
        zero_outs=zero_outs, nc=nc,
        devices=devices,
        pool=ThreadPoolExecutor(NCORES),
    )

    def rep0(a):
        """Shard array: real data on core 0, device-created zeros on 1-7.
        The NEFF AllReduce-replicates at NeuronLink speed."""
        z = jnp.zeros((NCORES * a.shape[0],) + a.shape[1:], a.dtype, device=spec)
        shards = sorted(z.addressable_shards, key=lambda s: s.index[0].start)
        real0 = jax.device_put(a, devices[0])
        return jax.make_array_from_single_device_arrays(
            z.shape, spec, [real0] + [s.data for s in shards[1:]]
        )

    # Upload core-0-only DFT matrices, then run the aux replicator NEFF
    # once; its outputs (full per-core replicas) stay device-resident and
    # feed the main NEFF directly, removing 4 AllReduces from every call.
    from concourse import bass2jax as b2j

    rep_nc = _build_replicator()
    rjf, rin, rout, _, rzeros = _make_runner(rep_nc)
    rargs = {"CF": rep0(CF), "SF": rep0(SF), "IC": rep0(IC), "IS": rep0(IS)}
    rlist = [rargs[n] for n in rin]
    rz = [
        jax.device_put(
            np.zeros((NCORES * z.shape[0],) + z.shape[1:], z.dtype), spec
        )
        for z in rzeros
    ]
    rfn = b2j.fast_dispatch_compile(
        lambda: rjf().lower(*rlist, *rz).compile()
    )
    routs = rfn(*rlist, *rz)
    jax.block_until_ready(routs)
    dft_dev = {n[:-1]: a for n, a in zip(rout, routs)}
    assert set(dft_dev) == {"CF", "SF", "IC", "IS"}, sorted(dft_dev)
    _STATE["dft"] = dft_dev
    _STATE["rep0"] = rep0
    t3 = time.perf_counter()

    # AOT-compile on the fast dispatch path, then warm with zero inputs
    from concourse import bass2jax as b2j

    q0 = jax.device_put(np.zeros((NCORES * S, F), np.float16), spec)
    it0 = jax.device_put(np.zeros((NCORES * F, K), np.float16), spec)
    ne0 = jax.device_put(np.zeros((NCORES * 1, K), np.float16), spec)
    arrs = {"q16": q0, "itemsT": it0, "negesqhalf": ne0, **dft_dev}
    args = [arrs[n] for n in in_names]
    zeros = [
        jax.device_put(
            np.zeros((NCORES * z.shape[0],) + z.shape[1:], z.dtype), spec
        )
        for z in zero_outs
    ]
    fn = b2j.fast_dispatch_compile(
        lambda: jit_factory().lower(*args, *zeros).compile()
    )
    _STATE["fn"] = fn
    out = fn(*args, *zeros)
    jax.block_until_ready(out)
    del out
    t4 = time.perf_counter()
    if os.environ.get("KERNEL_DEBUG"):
        print(
            f"[kernel init] build={t1-t0:.1f}s runner={t2-t1:.1f}s "
            f"dft_upload={t3-t2:.1f}s warmup={t4-t3:.1f}s",
            flush=True,
        )


def _call_device(q16_dev, itT_dev, nesq_dev, zeros=None):
    import jax

    st = _STATE
    arrs = {
        "q16": q16_dev,
        "itemsT": itT_dev,
        "negesqhalf": nesq_dev,
        **st["dft"],
    }
    args = [arrs[n] for n in st["in_names"]]
    if zeros is None:
        zeros = [
            jax.device_put(
                np.zeros((NCORES * z.shape[0],) + z.shape[1:], z.dtype), st["spec"]
            )
            for z in st["zero_outs"]
        ]
    outs = st["fn"](*args, *zeros)
    return outs


def kernel(queries: np.ndarray, items: np.ndarray) -> np.ndarray:
    import traceback

    try:
        _ensure_init()
    except Exception:
        traceback.print_exc()
        _STATE["fallback"] = True
    if _STATE.get("fallback"):
        return _host_kernel(queries, items)
    try:
        return _kernel_device(queries, items)
    except Exception:
        traceback.print_exc()
    try:
        # one retry: transient transfer/exec hiccups shouldn't demote us
        # to the slow host path permanently
        return _kernel_device(queries, items)
    except Exception:
        traceback.print_exc()
        _STATE["fallback"] = True
        return _host_kernel(queries, items)


def _kernel_device(queries: np.ndarray, items: np.ndarray) -> np.ndarray:
    import jax

    dbg = os.environ.get("KERNEL_DEBUG")
    t0 = time.perf_counter()
    st = _STATE
    spec = st["spec"]
    devices = st["devices"]
    pool = st["pool"]

    # Issue small tensors first (their handshakes shouldn't tail behind
    # the 32MB q transfer), then q per-batch so chunk 0's transfer
    # overlaps the remaining fp32->fp16 conversions.
    zeros = [
        jax.device_put(
            np.zeros((NCORES * z.shape[0],) + z.shape[1:], z.dtype), spec
        )
        for z in st["zero_outs"]
    ]
    items_key = hashlib.sha1(np.ascontiguousarray(items)).hexdigest()
    cached = st.get("items_cache")
    if cached is not None and cached[0] == items_key:
        itT_dev, ne_dev = cached[1], cached[2]
    else:
        itemsf = np.asarray(items, np.float32)
        itT = np.ascontiguousarray(itemsf.T).astype(np.float16)
        esq = np.sum(itemsf * itemsf, axis=1)
        nesq = (-0.5 * esq).astype(np.float16)[None, :]
        itT_dev = st["rep0"](itT)  # core 0 only; NEFF AllReduce replicates
        ne_dev = jax.device_put(
            np.broadcast_to(nesq, (NCORES,) + nesq.shape).reshape(NCORES, K), spec
        )
        st["items_cache"] = (items_key, itT_dev, ne_dev)
    q = np.asarray(queries)
    shards = []
    for b in range(NCORES):
        qb = np.asarray(q[b], np.float32).astype(np.float16)
        shards.append(jax.device_put(qb, devices[b]))
    q_dev = jax.make_array_from_single_device_arrays(
        (NCORES * S, F), spec, shards
    )
    t1 = time.perf_counter()
    outs = _call_device(q_dev, itT_dev, ne_dev, zeros)
    out_shards = sorted(outs[0].addressable_shards, key=lambda s: s.index[0].start)
    for sh in out_shards:
        try:
            sh.data.copy_to_host_async()
        except Exception:
            pass
    t2 = time.perf_counter()
    parts = list(pool.map(lambda s: np.asarray(s.data), out_shards))
    loss = np.concatenate(parts, axis=0).reshape(NCORES, S)
    t3 = time.perf_counter()
    if dbg:
        print(
            f"[kernel] issue={t1-t0:.3f}s call={t2-t1:.3f}s "
            f"fetch={t3-t2:.3f}s total={t3-t0:.3f}s",
            flush=True,
        )
    return loss


if not os.environ.get("KERNEL_NO_INIT"):
    try:
        _ensure_init()
    except Exception:
        import traceback

        traceback.print_exc()
        _STATE["fallback"] = True
